# revision 8
# baseline (speedup 1.0000x reference)
"""Trainium2 Bass kernel for nn_BidirRecurrentModel (2-layer bidirectional GRU).

Structure (measured 1004us baseline -> ~343us):
  * Gate PSUM split into two banks per layer-step:
      bank A [128,256] = [u | xr]   (read only by VectorE)
      bank B [128,256] = [r | z]    (read only by ScalarE)
    so ACT and DVE chain ops never touch the same PSUM bank (bank
    collisions with PE writes force Tile to serialize otherwise).
  * All matmuls emitted round-robin across the 4 tile_position column
    groups — consecutive MMs in the same 32-col group serialize on that
    PE sub-array, different groups stream concurrently.
  * h-recurrence streamed as (u,r) first then z, so sigmoid(r) starts
    as early as possible.
  * Layer-2 stream lags layer-1 by TWO steps so layer-2's x-projections
    (contracting hT1) prefetch fully off the critical path.
  * Layer-1 work emitted FIRST each tau: the engine queues are strict
    FIFO, so the serial-bottleneck loop's ops must sit at the head or
    layer-2's waiting ops head-of-line-block them (this alone was ~2x).
  * Input DMAs chunked and priority-ordered (layer-1 weights + first xT
    chunk first, tail-only tensors last) so step 0 isn't gated on the
    full ~14MB transfer; A banks double-buffered in PSUM.
  * Chain tail in bf16 (2x DVE mode), transpose output copied once.

Layouts (B=32, T=128, I=H=O=512):
  packed [128,128]: row 32k+b <-> (chunk k, batch b), col j = dim-in-chunk
  hT     [128,128]: hT[p, 32k+b] = h[b, 128k+p]
"""

import numpy as np

import concourse.bass as bass
import concourse.mybir as mybir
import concourse.tile as tile
from concourse import bacc
from concourse.bass_utils import run_bass_kernel_spmd

F32 = mybir.dt.float32
BF16 = mybir.dt.bfloat16
AF = mybir.ActivationFunctionType

B, T, I, H, O = 32, 128, 512, 512, 512
KC = 4
NCORES = 8

import ml_dtypes
BFNP = ml_dtypes.bfloat16


def _to_bf16(a):
    return np.asarray(a, np.float32).astype(BFNP)


def _pack_vec(v):
    """[512] -> packed replicated [128, 128]: out[32k+b, j] = v[128k+j]."""
    v = np.asarray(v, np.float32).reshape(KC, 128)
    out = np.repeat(v[:, None, :], B, axis=1)
    return out.reshape(128, 128)


def _blocks128(W):
    """W [512, 512] -> [128, KC*4*128]: block (k, gc) = W[128k:.., 128gc:..]."""
    out = np.zeros((128, KC * 4 * 128), np.float32)
    for k in range(KC):
        for gc in range(4):
            out[:, (k * 4 + gc) * 128 : (k * 4 + gc) * 128 + 128] = W[
                128 * k : 128 * k + 128, 128 * gc : 128 * gc + 128
            ]
    return _to_bf16(out)


def _blocks256(W0, W1):
    """Two [512,512] -> [128, KC*4*256]: block (k,gc) = [W0[k,gc] | W1[k,gc]]."""
    out = np.zeros((128, KC * 4 * 256), np.float32)
    for k in range(KC):
        for gc in range(4):
            base = (k * 4 + gc) * 256
            out[:, base : base + 128] = W0[128 * k : 128 * k + 128, 128 * gc : 128 * gc + 128]
            out[:, base + 128 : base + 256] = W1[128 * k : 128 * k + 128, 128 * gc : 128 * gc + 128]
    return _to_bf16(out)


def _pack_T(xt):
    """[B, 512] -> stationary image [128, 128]: out[p, 32k+b] = xt[b, 128k+p]."""
    a = np.asarray(xt, np.float32).T.reshape(KC, 128, B)  # [k, p, b]
    return a.transpose(1, 0, 2).reshape(128, KC * B)


def _cell_tiles(Wxh, bxh, Whh, bhh, Wxr, bxr, Whr, bhr, with_h):
    """Host tiles for one GRU cell. z = first H cols of Whh/Wxh, r = second."""
    Wz, Wr = np.asarray(Whh, np.float32)[:, :H], np.asarray(Whh, np.float32)[:, H:]
    Wxz, Wxr_h = np.asarray(Wxh, np.float32)[:, :H], np.asarray(Wxh, np.float32)[:, H:]
    bz = np.asarray(bxh, np.float32)[:H] + np.asarray(bhh, np.float32)[:H]
    br = np.asarray(bxh, np.float32)[H:] + np.asarray(bhh, np.float32)[H:]
    t = {}
    t["cgA"] = _to_bf16(
        np.concatenate([_pack_vec(np.asarray(bhr, np.float32)), _pack_vec(np.asarray(bxr, np.float32))], axis=1)
    )  # [128, 256] = [u-bias | xr-bias]
    t["cgB"] = _to_bf16(np.concatenate([_pack_vec(br), _pack_vec(bz)], axis=1))  # [r | z]
    t["wxA"] = _blocks128(np.asarray(Wxr, np.float32))        # xr  -> A[128:256]
    t["wxB"] = _blocks256(Wxr_h, Wxz)                          # r|z -> B[0:256]
    if with_h:
        t["wgA"] = _blocks128(np.asarray(Whr, np.float32))     # u -> A[0:128]
        t["wgBr"] = _blocks128(Wr)                              # r -> B[0:128]
        t["wgBz"] = _blocks128(Wz)                              # z -> B[128:256]
    return t


def prepare_inputs(x, Wxh, bxh, Whh, bhh, Wxr, bxr, Whr, bhr, Wfc, bfc):
    h = {}
    xT = np.zeros((128, T * 128), np.float32)
    for t in range(T):
        xT[:, t * 128 : (t + 1) * 128] = _pack_T(x[:, t, :])
    h["xT"] = _to_bf16(xT)
    h["xrevT"] = _to_bf16(_pack_T(x[:, T - 1, :]))

    for l in range(2):
        fw = _cell_tiles(Wxh[l, 0], bxh[l, 0], Whh[l, 0], bhh[l, 0],
                         Wxr[l, 0], bxr[l, 0], Whr[l, 0], bhr[l, 0], with_h=True)
        for k, v in fw.items():
            h[f"{k}{l}"] = v
        rv = _cell_tiles(Wxh[l, 1], bxh[l, 1], Whh[l, 1], bhh[l, 1],
                         Wxr[l, 1], bxr[l, 1], Whr[l, 1], bhr[l, 1], with_h=False)
        for k, v in rv.items():
            h[f"{k}{l}r"] = v

    wfc = np.zeros((128, 8 * O), np.float32)
    for kk in range(8):
        wfc[:, kk * O : (kk + 1) * O] = np.asarray(Wfc, np.float32)[128 * kk : 128 * (kk + 1), :]
    h["wfc"] = _to_bf16(wfc)
    h["bfcrep"] = np.repeat(np.asarray(bfc, np.float32)[None, :], B, axis=0)
    h["ideye"] = _to_bf16(np.eye(128, dtype=np.float32))
    return h


def build_program(host, n_steps=T, split_waits=False):
    nc = bacc.Bacc("TRN2", target_bir_lowering=False, debug=False, num_devices=NCORES)
    dram = {}
    for name, arr in host.items():
        dt = BF16 if arr.dtype == BFNP else F32
        dram[name] = nc.dram_tensor(name, list(arr.shape), dt, kind="ExternalInput")
    out_d = nc.dram_tensor("out", [B, O], F32, kind="ExternalOutput")

    with tile.TileContext(nc) as tc:
        _emit(tc, dram, out_d, n_steps)
    nc.compile()
    if split_waits:
        _split_multi_waits(nc)
    return nc


def _split_multi_waits(nc):
    n_nop = 0
    for fn in nc.m.functions:
        for blk in fn.blocks:
            out = []
            changed = False
            for inst in blk.instructions:
                si = inst.sync_info
                if si is not None and si.on_wait and len(si.on_wait) > 1:
                    waits = list(si.on_wait)
                    for w in waits[:-1]:
                        n_nop += 1
                        out.append(
                            mybir.InstNoOp(
                                name=f"waitnop-{n_nop}",
                                engine=inst.engine,
                                ins=[],
                                outs=[],
                                sync_info=mybir.SyncInfo(on_wait=[w], on_update=[]),
                            )
                        )
                    inst = inst.__replace__(
                        sync_info=mybir.SyncInfo(
                            on_wait=[waits[-1]], on_update=list(si.on_update or [])
                        )
                    )
                    changed = True
                out.append(inst)
            if changed:
                blk.instructions = out


def _emit(tc, dram, out_d, n_steps):
    nc = tc.nc
    from contextlib import ExitStack

    ctx = ExitStack()
    consts = ctx.enter_context(tc.tile_pool(name="consts", bufs=1))
    hpool = ctx.enter_context(tc.tile_pool(name="h", bufs=3))
    chain = ctx.enter_context(tc.tile_pool(name="chain", bufs=2))
    # A banks double-buffered (their reads release latest in the chain);
    # FC rides tag A0 at the very end. 2*2 + 2*1 + 2*1 = 8 PSUM banks.
    psumsA = ctx.enter_context(tc.tile_pool(name="psumA", bufs=2, space="PSUM"))
    psums = ctx.enter_context(tc.tile_pool(name="psum", bufs=1, space="PSUM"))

    sb = {}

    def load(name, n_chunks=1):
        """Chunked prioritized DRAM->SBUF load: compute that depends only on
        early columns starts after the first chunk, not the whole tensor."""
        d = dram[name]
        t = consts.tile(list(d.shape), d.dtype, name=f"sb_{name}", tag=name)
        cols = d.shape[-1]
        cw = cols // n_chunks
        for c in range(n_chunks):
            sl = slice(c * cw, (c + 1) * cw)
            nc.sync.dma_start(t[:, sl], d.ap()[:, sl])
        sb[name] = t

    # priority order: what gates step 0 first, tail-only tensors last
    load("ideye")
    for nm in ("cgA0", "cgB0"):
        load(nm)
    for nm in ("wxA0", "wgA0", "wgBr0", "wgBz0"):
        load(nm, 2)
    load("wxB0", 4)
    load("xT", 16)
    for nm in ("cgA1", "cgB1"):
        load(nm)
    for nm in ("wxA1", "wgA1", "wgBr1", "wgBz1"):
        load(nm, 2)
    load("wxB1", 4)
    for nm in ("xrevT", "cgA0r", "cgB0r", "wxA0r", "cgA1r", "cgB1r", "wxA1r",
               "bfcrep"):
        load(nm)
    load("wxB0r", 2)
    load("wxB1r", 2)
    load("wfc", 4)

    ideye = sb["ideye"]

    def x_phase(l, sfx, xstat, xoff, with_h):
        """Emit consts + x-projection MMs into fresh A/B banks."""
        A = psumsA.tile([128, 256], F32, name=f"A{l}{sfx}", tag=f"A{l}")
        Bk = psums.tile([128, 256], F32, name=f"B{l}{sfx}", tag=f"B{l}")
        cgA, cgB = sb[f"cgA{l}{sfx}"], sb[f"cgB{l}{sfx}"]
        wxA, wxB = sb[f"wxA{l}{sfx}"], sb[f"wxB{l}{sfx}"]
        for gc in range(4):
            o = slice(32 * gc, 32 * gc + 32)
            tp = (0, 32 * gc)
            nc.tensor.matmul(A[o, 0:256], ideye[:, o], cgA[:, :], start=True,
                             stop=False, tile_position=tp)
            nc.tensor.matmul(Bk[o, 0:256], ideye[:, o], cgB[:, :], start=True,
                             stop=False, tile_position=tp)
        last = not with_h
        for k in range(KC):
            st = xstat[:, xoff + 32 * k : xoff + 32 * k + 32]
            for gc in range(4):
                o = slice(32 * gc, 32 * gc + 32)
                tp = (0, 32 * gc)
                stp = last and k == KC - 1
                nc.tensor.matmul(A[o, 128:256], st,
                                 wxA[:, (k * 4 + gc) * 128 : (k * 4 + gc) * 128 + 128],
                                 start=False, stop=stp, tile_position=tp)
                nc.tensor.matmul(Bk[o, 0:256], st,
                                 wxB[:, (k * 4 + gc) * 256 : (k * 4 + gc) * 256 + 256],
                                 start=False, stop=stp, tile_position=tp)
        return A, Bk

    def h_phase(l, A, Bk, hTp):
        """u,r first (so sigmoid(r) starts early), then z."""
        wgA, wgBr, wgBz = sb[f"wgA{l}"], sb[f"wgBr{l}"], sb[f"wgBz{l}"]
        for k in range(KC):
            st = hTp[:, 32 * k : 32 * k + 32]
            for gc in range(4):
                o = slice(32 * gc, 32 * gc + 32)
                tp = (0, 32 * gc)
                blk = slice((k * 4 + gc) * 128, (k * 4 + gc) * 128 + 128)
                nc.tensor.matmul(A[o, 0:128], st, wgA[:, blk], start=False,
                                 stop=(k == KC - 1), tile_position=tp)
                nc.tensor.matmul(Bk[o, 0:128], st, wgBr[:, blk], start=False,
                                 stop=False, tile_position=tp)
        for k in range(KC):
            st = hTp[:, 32 * k : 32 * k + 32]
            for gc in range(4):
                o = slice(32 * gc, 32 * gc + 32)
                tp = (0, 32 * gc)
                blk = slice((k * 4 + gc) * 128, (k * 4 + gc) * 128 + 128)
                nc.tensor.matmul(Bk[o, 128:256], st, wgBz[:, blk], start=False,
                                 stop=(k == KC - 1), tile_position=tp)

    def chain_phase(l, tag, A, Bk, hn_prev):
        rs = chain.tile([128, 128], F32, name=f"rs{tag}", tag=f"rs{tag}")
        nc.scalar.activation(rs[:], Bk[:, 0:128], AF.Sigmoid)
        cs = chain.tile([128, 128], BF16, name=f"cs{tag}", tag=f"cs{tag}")
        nc.scalar.activation(cs[:], Bk[:, 128:256], AF.Sigmoid, scale=-1.0)
        if hn_prev is not None:
            zs = chain.tile([128, 128], BF16, name=f"zs{tag}", tag=f"zs{tag}")
            nc.scalar.activation(zs[:], Bk[:, 128:256], AF.Sigmoid)
        v0 = chain.tile([128, 128], F32, name=f"v0{tag}", tag=f"v0{tag}")
        nc.vector.tensor_mul(v0[:], A[:, 0:128], rs[:])
        v1 = chain.tile([128, 128], F32, name=f"v1{tag}", tag=f"v1{tag}")
        nc.vector.tensor_add(v1[:], v0[:], A[:, 128:256])
        if hn_prev is not None:
            p = chain.tile([128, 128], BF16, name=f"p{tag}", tag=f"p{tag}")
            nc.vector.tensor_mul(p[:], zs[:], hn_prev[:])
        g = chain.tile([128, 128], BF16, name=f"g{tag}", tag=f"g{tag}")
        nc.scalar.activation(g[:], v1[:], AF.Tanh)
        q = chain.tile([128, 128], BF16, name=f"q{tag}", tag=f"q{tag}")
        nc.vector.tensor_mul(q[:], cs[:], g[:])
        if hn_prev is None:
            return q
        hn = hpool.tile([128, 128], BF16, name=f"hn{l}", tag=f"hn{l}")
        nc.vector.tensor_add(hn[:], p[:], q[:])
        return hn

    def t_phase(l, tag, hn):
        Tp = psums.tile([128, 128], BF16, name=f"T{tag}", tag=f"T{l}")
        nc.tensor.transpose(Tp[:], hn[:], ideye[:])
        hT = hpool.tile([128, 128], BF16, name=f"hT{tag}", tag=f"hT{l}")
        nc.scalar.copy(hT[:], Tp[:])
        return hT

    # ---------------- forward recurrence ----------------
    # L1 (layer 0 fwd) at step t1 = tau; L2 (layer 1 fwd) lags two steps.
    AB1 = {}
    AB2 = {}
    hT1 = {}  # step -> tile (kept ~2 taus)
    hT2 = {}
    hn1 = hn2 = None

    AB1[0] = x_phase(0, "", sb["xT"], 0, with_h=False)

    for tau in range(n_steps + 2):
        t1 = tau
        t2 = tau - 2
        # L1 first everywhere: it is the serial bottleneck loop (its state
        # feeds both its own next step and L2's input stream).
        if t1 < n_steps and t1 > 0:
            h_phase(0, *AB1[t1], hT1[t1 - 1])
        if 0 <= t2 < n_steps and t2 > 0:
            h_phase(1, *AB2[t2], hT2[t2 - 1])
        if t1 < n_steps:
            hn1 = chain_phase(0, "1", *AB1[t1], hn1 if t1 > 0 else None)
        if t1 + 1 < n_steps:
            AB1[t1 + 1] = x_phase(0, "", sb["xT"], (t1 + 1) * 128, with_h=(t1 + 1) > 0)
        if 0 <= t2 < n_steps:
            hn2 = chain_phase(1, "2", *AB2[t2], hn2 if t2 > 0 else None)
        if t1 < n_steps:
            hT1[t1] = t_phase(0, f"1_{t1}", hn1)
            hT1.pop(t1 - 3, None)
        if 0 <= t2 + 1 < n_steps:
            AB2[t2 + 1] = x_phase(1, "", hT1[t2 + 1], 0, with_h=(t2 + 1) > 0)
        if 0 <= t2 < n_steps:
            hT2[t2] = t_phase(1, f"2_{t2}", hn2)
            hT2.pop(t2 - 2, None)
        AB1.pop(t1, None)
        AB2.pop(t2, None)

    # ---------------- reverse stream: one step per layer, h0 = 0 ----------------
    Ar, Br = x_phase(0, "r", sb["xrevT"], 0, with_h=False)
    q1r = chain_phase(0, "r1", Ar, Br, None)
    h1rT = t_phase(0, "r1", q1r)
    Ar2, Br2 = x_phase(1, "r", h1rT, 0, with_h=False)
    q2r = chain_phase(1, "r2", Ar2, Br2, None)
    h2rT = t_phase(1, "r2", q2r)

    # ---------------- final FC ----------------
    hT2_last = hT2[n_steps - 1]
    FCp = psumsA.tile([B, O], F32, name="FCp", tag="A0")
    for kk in range(8):
        st = hT2_last[:, 32 * kk : 32 * kk + 32] if kk < 4 else h2rT[:, 32 * (kk - 4) : 32 * (kk - 4) + 32]
        nc.tensor.matmul(FCp[:, :], st, sb["wfc"][:, kk * O : (kk + 1) * O],
                         start=(kk == 0), stop=(kk == 7))
    outsb = consts.tile([B, O], F32, name="outsb", tag="outsb")
    nc.vector.tensor_add(outsb[:], FCp[:], sb["bfcrep"][:])
    nc.sync.dma_start(out_d.ap(), outsb[:])
    ctx.close()


_CACHE = {}


def _run(host, trace=False, n_steps=T):
    key = ("prog", n_steps)
    if key not in _CACHE:
        _CACHE[key] = build_program(host, n_steps)
    nc = _CACHE[key]
    in_map = {k: np.ascontiguousarray(v) for k, v in host.items()}
    res = run_bass_kernel_spmd(
        nc, [in_map] * NCORES, core_ids=list(range(NCORES)), trace=trace
    )
    return res


def kernel(**inputs):
    host = prepare_inputs(**{k: np.asarray(v) for k, v in inputs.items()})
    res = _run(host, trace=False)
    return np.asarray(res.results[0]["out"], np.float32)


# revision 10
# speedup vs baseline: 1.2790x; 1.2790x over previous
"""Trainium2 Bass kernel for nn_BidirRecurrentModel (2-layer bidirectional GRU).

Structure (measured 1004us baseline -> ~343us):
  * Gate PSUM split into two banks per layer-step:
      bank A [128,256] = [u | xr]   (read only by VectorE)
      bank B [128,256] = [r | z]    (read only by ScalarE)
    so ACT and DVE chain ops never touch the same PSUM bank (bank
    collisions with PE writes force Tile to serialize otherwise).
  * All matmuls emitted round-robin across the 4 tile_position column
    groups — consecutive MMs in the same 32-col group serialize on that
    PE sub-array, different groups stream concurrently.
  * h-recurrence streamed as (u,r) first then z, so sigmoid(r) starts
    as early as possible.
  * Layer-2 stream lags layer-1 by TWO steps so layer-2's x-projections
    (contracting hT1) prefetch fully off the critical path.
  * Layer-1 work emitted FIRST each tau: the engine queues are strict
    FIFO, so the serial-bottleneck loop's ops must sit at the head or
    layer-2's waiting ops head-of-line-block them (this alone was ~2x).
  * Input DMAs chunked and priority-ordered (layer-1 weights + first xT
    chunk first, tail-only tensors last) so step 0 isn't gated on the
    full ~14MB transfer; A banks double-buffered in PSUM.
  * Chain tail in bf16 (2x DVE mode), transpose output copied once.

Layouts (B=32, T=128, I=H=O=512):
  packed [128,128]: row 32k+b <-> (chunk k, batch b), col j = dim-in-chunk
  hT     [128,128]: hT[p, 32k+b] = h[b, 128k+p]
"""

import numpy as np

import concourse.bass as bass
import concourse.mybir as mybir
import concourse.tile as tile
from concourse import bacc
from concourse.bass_utils import run_bass_kernel_spmd

F32 = mybir.dt.float32
BF16 = mybir.dt.bfloat16
AF = mybir.ActivationFunctionType

B, T, I, H, O = 32, 128, 512, 512, 512
KC = 4
NCORES = 8

import ml_dtypes
BFNP = ml_dtypes.bfloat16


def _to_bf16(a):
    return np.asarray(a, np.float32).astype(BFNP)


def _pack_vec(v):
    """[512] -> packed replicated [128, 128]: out[32k+b, j] = v[128k+j]."""
    v = np.asarray(v, np.float32).reshape(KC, 128)
    out = np.repeat(v[:, None, :], B, axis=1)
    return out.reshape(128, 128)


def _blocks128(W):
    """W [512, 512] -> [128, KC*4*128]: block (k, gc) = W[128k:.., 128gc:..]."""
    out = np.zeros((128, KC * 4 * 128), np.float32)
    for k in range(KC):
        for gc in range(4):
            out[:, (k * 4 + gc) * 128 : (k * 4 + gc) * 128 + 128] = W[
                128 * k : 128 * k + 128, 128 * gc : 128 * gc + 128
            ]
    return _to_bf16(out)


def _blocks256(W0, W1):
    """Two [512,512] -> [128, KC*4*256]: block (k,gc) = [W0[k,gc] | W1[k,gc]]."""
    out = np.zeros((128, KC * 4 * 256), np.float32)
    for k in range(KC):
        for gc in range(4):
            base = (k * 4 + gc) * 256
            out[:, base : base + 128] = W0[128 * k : 128 * k + 128, 128 * gc : 128 * gc + 128]
            out[:, base + 128 : base + 256] = W1[128 * k : 128 * k + 128, 128 * gc : 128 * gc + 128]
    return _to_bf16(out)


def _pack_T(xt):
    """[B, 512] -> stationary image [128, 128]: out[p, 32k+b] = xt[b, 128k+p]."""
    a = np.asarray(xt, np.float32).T.reshape(KC, 128, B)  # [k, p, b]
    return a.transpose(1, 0, 2).reshape(128, KC * B)


def _cell_tiles(Wxh, bxh, Whh, bhh, Wxr, bxr, Whr, bhr, with_h):
    """Host tiles for one GRU cell. z = first H cols of Whh/Wxh, r = second."""
    Wz, Wr = np.asarray(Whh, np.float32)[:, :H], np.asarray(Whh, np.float32)[:, H:]
    Wxz, Wxr_h = np.asarray(Wxh, np.float32)[:, :H], np.asarray(Wxh, np.float32)[:, H:]
    bz = np.asarray(bxh, np.float32)[:H] + np.asarray(bhh, np.float32)[:H]
    br = np.asarray(bxh, np.float32)[H:] + np.asarray(bhh, np.float32)[H:]
    t = {}
    t["cgA"] = _to_bf16(
        np.concatenate([_pack_vec(np.asarray(bhr, np.float32)), _pack_vec(np.asarray(bxr, np.float32))], axis=1)
    )  # [128, 256] = [u-bias | xr-bias]
    t["cgB"] = _to_bf16(np.concatenate([_pack_vec(br), _pack_vec(bz)], axis=1))  # [r | z]
    t["wxA"] = _blocks128(np.asarray(Wxr, np.float32))        # xr  -> A[128:256]
    t["wxB"] = _blocks256(Wxr_h, Wxz)                          # r|z -> B[0:256]
    if with_h:
        t["wgA"] = _blocks128(np.asarray(Whr, np.float32))     # u -> A[0:128]
        t["wgBr"] = _blocks128(Wr)                              # r -> B[0:128]
        t["wgBz"] = _blocks128(Wz)                              # z -> B[128:256]
    return t


def prepare_inputs(x, Wxh, bxh, Whh, bhh, Wxr, bxr, Whr, bhr, Wfc, bfc):
    h = {}
    xT = np.zeros((128, T * 128), np.float32)
    for t in range(T):
        xT[:, t * 128 : (t + 1) * 128] = _pack_T(x[:, t, :])
    h["xT"] = _to_bf16(xT)
    h["xrevT"] = _to_bf16(_pack_T(x[:, T - 1, :]))

    for l in range(2):
        fw = _cell_tiles(Wxh[l, 0], bxh[l, 0], Whh[l, 0], bhh[l, 0],
                         Wxr[l, 0], bxr[l, 0], Whr[l, 0], bhr[l, 0], with_h=True)
        for k, v in fw.items():
            h[f"{k}{l}"] = v
        rv = _cell_tiles(Wxh[l, 1], bxh[l, 1], Whh[l, 1], bhh[l, 1],
                         Wxr[l, 1], bxr[l, 1], Whr[l, 1], bhr[l, 1], with_h=False)
        for k, v in rv.items():
            h[f"{k}{l}r"] = v

    wfc = np.zeros((128, 8 * O), np.float32)
    for kk in range(8):
        wfc[:, kk * O : (kk + 1) * O] = np.asarray(Wfc, np.float32)[128 * kk : 128 * (kk + 1), :]
    h["wfc"] = _to_bf16(wfc)
    h["bfcrep"] = np.repeat(np.asarray(bfc, np.float32)[None, :], B, axis=0)
    h["ideye"] = _to_bf16(np.eye(128, dtype=np.float32))
    return h


def build_program(host, n_steps=T, split_waits=False):
    nc = bacc.Bacc("TRN2", target_bir_lowering=False, debug=False, num_devices=NCORES)
    dram = {}
    for name, arr in host.items():
        dt = BF16 if arr.dtype == BFNP else F32
        dram[name] = nc.dram_tensor(name, list(arr.shape), dt, kind="ExternalInput")
    out_d = nc.dram_tensor("out", [B, O], F32, kind="ExternalOutput")

    with tile.TileContext(nc) as tc:
        _emit(tc, dram, out_d, n_steps)
    nc.compile()
    if split_waits:
        _split_multi_waits(nc)
    return nc


def _split_multi_waits(nc):
    n_nop = 0
    for fn in nc.m.functions:
        for blk in fn.blocks:
            out = []
            changed = False
            for inst in blk.instructions:
                si = inst.sync_info
                if si is not None and si.on_wait and len(si.on_wait) > 1:
                    waits = list(si.on_wait)
                    for w in waits[:-1]:
                        n_nop += 1
                        out.append(
                            mybir.InstNoOp(
                                name=f"waitnop-{n_nop}",
                                engine=inst.engine,
                                ins=[],
                                outs=[],
                                sync_info=mybir.SyncInfo(on_wait=[w], on_update=[]),
                            )
                        )
                    inst = inst.__replace__(
                        sync_info=mybir.SyncInfo(
                            on_wait=[waits[-1]], on_update=list(si.on_update or [])
                        )
                    )
                    changed = True
                out.append(inst)
            if changed:
                blk.instructions = out


def _emit(tc, dram, out_d, n_steps):
    nc = tc.nc
    from contextlib import ExitStack

    ctx = ExitStack()
    consts = ctx.enter_context(tc.tile_pool(name="consts", bufs=1))
    hpool = ctx.enter_context(tc.tile_pool(name="h", bufs=4))
    chain = ctx.enter_context(tc.tile_pool(name="chain", bufs=3))
    # A banks double-buffered (their reads release latest in the chain);
    # FC rides tag A0 at the very end. 2*2 + 2*1 + 2*1 = 8 PSUM banks.
    psumsA = ctx.enter_context(tc.tile_pool(name="psumA", bufs=2, space="PSUM"))
    psums = ctx.enter_context(tc.tile_pool(name="psum", bufs=1, space="PSUM"))

    sb = {}

    def load(name, n_chunks=1):
        """Chunked prioritized DRAM->SBUF load: compute that depends only on
        early columns starts after the first chunk, not the whole tensor."""
        d = dram[name]
        t = consts.tile(list(d.shape), d.dtype, name=f"sb_{name}", tag=name)
        cols = d.shape[-1]
        cw = cols // n_chunks
        for c in range(n_chunks):
            sl = slice(c * cw, (c + 1) * cw)
            nc.sync.dma_start(t[:, sl], d.ap()[:, sl])
        sb[name] = t

    # priority order: what gates step 0 first, tail-only tensors last
    load("ideye")
    for nm in ("cgA0", "cgB0"):
        load(nm)
    for nm in ("wxA0", "wgA0", "wgBr0", "wgBz0"):
        load(nm, 2)
    load("wxB0", 4)
    load("xT", 16)
    for nm in ("cgA1", "cgB1"):
        load(nm)
    for nm in ("wxA1", "wgA1", "wgBr1", "wgBz1"):
        load(nm, 2)
    load("wxB1", 4)
    for nm in ("xrevT", "cgA0r", "cgB0r", "wxA0r", "cgA1r", "cgB1r", "wxA1r",
               "bfcrep"):
        load(nm)
    load("wxB0r", 2)
    load("wxB1r", 2)
    load("wfc", 4)

    ideye = sb["ideye"]

    def x_phase(l, sfx, xstat, xoff, with_h):
        """Emit consts + x-projection MMs into fresh A/B banks."""
        A = psumsA.tile([128, 256], F32, name=f"A{l}{sfx}", tag=f"A{l}")
        Bk = psums.tile([128, 256], F32, name=f"B{l}{sfx}", tag=f"B{l}")
        cgA, cgB = sb[f"cgA{l}{sfx}"], sb[f"cgB{l}{sfx}"]
        wxA, wxB = sb[f"wxA{l}{sfx}"], sb[f"wxB{l}{sfx}"]
        for gc in range(4):
            o = slice(32 * gc, 32 * gc + 32)
            tp = (0, 32 * gc)
            nc.tensor.matmul(A[o, 0:256], ideye[:, o], cgA[:, :], start=True,
                             stop=False, tile_position=tp)
            nc.tensor.matmul(Bk[o, 0:256], ideye[:, o], cgB[:, :], start=True,
                             stop=False, tile_position=tp)
        last = not with_h
        for k in range(KC):
            st = xstat[:, xoff + 32 * k : xoff + 32 * k + 32]
            for gc in range(4):
                o = slice(32 * gc, 32 * gc + 32)
                tp = (0, 32 * gc)
                stp = last and k == KC - 1
                nc.tensor.matmul(A[o, 128:256], st,
                                 wxA[:, (k * 4 + gc) * 128 : (k * 4 + gc) * 128 + 128],
                                 start=False, stop=stp, tile_position=tp)
                nc.tensor.matmul(Bk[o, 0:256], st,
                                 wxB[:, (k * 4 + gc) * 256 : (k * 4 + gc) * 256 + 256],
                                 start=False, stop=stp, tile_position=tp)
        return A, Bk

    def h_phase(l, A, Bk, hTp):
        """u,r first (so sigmoid(r) starts early), then z."""
        wgA, wgBr, wgBz = sb[f"wgA{l}"], sb[f"wgBr{l}"], sb[f"wgBz{l}"]
        for k in range(KC):
            st = hTp[:, 32 * k : 32 * k + 32]
            for gc in range(4):
                o = slice(32 * gc, 32 * gc + 32)
                tp = (0, 32 * gc)
                blk = slice((k * 4 + gc) * 128, (k * 4 + gc) * 128 + 128)
                nc.tensor.matmul(A[o, 0:128], st, wgA[:, blk], start=False,
                                 stop=(k == KC - 1), tile_position=tp)
                nc.tensor.matmul(Bk[o, 0:128], st, wgBr[:, blk], start=False,
                                 stop=False, tile_position=tp)
        for k in range(KC):
            st = hTp[:, 32 * k : 32 * k + 32]
            for gc in range(4):
                o = slice(32 * gc, 32 * gc + 32)
                tp = (0, 32 * gc)
                blk = slice((k * 4 + gc) * 128, (k * 4 + gc) * 128 + 128)
                nc.tensor.matmul(Bk[o, 128:256], st, wgBz[:, blk], start=False,
                                 stop=(k == KC - 1), tile_position=tp)

    def chain_phase(l, tag, A, Bk, hn_prev):
        rs = chain.tile([128, 128], F32, name=f"rs{tag}", tag=f"rs{tag}")
        nc.scalar.activation(rs[:], Bk[:, 0:128], AF.Sigmoid)
        zs = chain.tile([128, 128], BF16, name=f"zs{tag}", tag=f"zs{tag}")
        nc.scalar.activation(zs[:], Bk[:, 128:256], AF.Sigmoid)
        v0 = chain.tile([128, 128], F32, name=f"v0{tag}", tag=f"v0{tag}")
        nc.vector.tensor_mul(v0[:], A[:, 0:128], rs[:])
        v1 = chain.tile([128, 128], BF16, name=f"v1{tag}", tag=f"v1{tag}")
        nc.vector.tensor_add(v1[:], v0[:], A[:, 128:256])
        if hn_prev is not None:
            p = chain.tile([128, 128], BF16, name=f"p{tag}", tag=f"p{tag}")
            nc.vector.tensor_mul(p[:], zs[:], hn_prev[:])
        g = chain.tile([128, 128], BF16, name=f"g{tag}", tag=f"g{tag}")
        nc.scalar.activation(g[:], v1[:], AF.Tanh)
        # qn = (zs - 1) * g = -(1-z)*g in ONE fused DVE op (drops the
        # sigmoid(-z) ACT op from the per-step chain entirely)
        qn = chain.tile([128, 128], BF16, name=f"q{tag}", tag=f"q{tag}")
        nc.vector.scalar_tensor_tensor(
            qn[:], zs[:], 1.0, g[:],
            mybir.AluOpType.subtract, mybir.AluOpType.mult,
        )
        if hn_prev is None:
            # h0 = 0: h' = (1-z)*g = -qn
            hn0 = chain.tile([128, 128], BF16, name=f"h0{tag}", tag=f"h0{tag}")
            nc.vector.tensor_scalar_mul(hn0[:], qn[:], -1.0)
            return hn0
        hn = hpool.tile([128, 128], BF16, name=f"hn{l}", tag=f"hn{l}")
        nc.vector.tensor_sub(hn[:], p[:], qn[:])
        return hn

    def t_phase(l, tag, hn):
        Tp = psums.tile([128, 128], BF16, name=f"T{tag}", tag=f"T{l}")
        nc.tensor.transpose(Tp[:], hn[:], ideye[:])
        hT = hpool.tile([128, 128], BF16, name=f"hT{tag}", tag=f"hT{l}")
        nc.scalar.copy(hT[:], Tp[:])
        return hT

    # ---------------- forward recurrence ----------------
    # L1 (layer 0 fwd) at step t1 = tau; L2 (layer 1 fwd) lags two steps.
    AB1 = {}
    AB2 = {}
    hT1 = {}  # step -> tile (kept ~2 taus)
    hT2 = {}
    hn1 = hn2 = None

    AB1[0] = x_phase(0, "", sb["xT"], 0, with_h=False)

    for tau in range(n_steps + 2):
        t1 = tau
        t2 = tau - 2
        # L1 first everywhere: it is the serial bottleneck loop (its state
        # feeds both its own next step and L2's input stream).
        if t1 < n_steps and t1 > 0:
            h_phase(0, *AB1[t1], hT1[t1 - 1])
        if 0 <= t2 < n_steps and t2 > 0:
            h_phase(1, *AB2[t2], hT2[t2 - 1])
        if t1 < n_steps:
            hn1 = chain_phase(0, "1", *AB1[t1], hn1 if t1 > 0 else None)
        if t1 + 1 < n_steps:
            AB1[t1 + 1] = x_phase(0, "", sb["xT"], (t1 + 1) * 128, with_h=(t1 + 1) > 0)
        if 0 <= t2 < n_steps:
            hn2 = chain_phase(1, "2", *AB2[t2], hn2 if t2 > 0 else None)
        if t1 < n_steps:
            hT1[t1] = t_phase(0, f"1_{t1}", hn1)
            hT1.pop(t1 - 3, None)
        if 0 <= t2 + 1 < n_steps:
            AB2[t2 + 1] = x_phase(1, "", hT1[t2 + 1], 0, with_h=(t2 + 1) > 0)
        if 0 <= t2 < n_steps:
            hT2[t2] = t_phase(1, f"2_{t2}", hn2)
            hT2.pop(t2 - 2, None)
        AB1.pop(t1, None)
        AB2.pop(t2, None)

    # ---------------- reverse stream: one step per layer, h0 = 0 ----------------
    Ar, Br = x_phase(0, "r", sb["xrevT"], 0, with_h=False)
    q1r = chain_phase(0, "r1", Ar, Br, None)
    h1rT = t_phase(0, "r1", q1r)
    Ar2, Br2 = x_phase(1, "r", h1rT, 0, with_h=False)
    q2r = chain_phase(1, "r2", Ar2, Br2, None)
    h2rT = t_phase(1, "r2", q2r)

    # ---------------- final FC ----------------
    hT2_last = hT2[n_steps - 1]
    FCp = psumsA.tile([B, O], F32, name="FCp", tag="A0")
    for kk in range(8):
        st = hT2_last[:, 32 * kk : 32 * kk + 32] if kk < 4 else h2rT[:, 32 * (kk - 4) : 32 * (kk - 4) + 32]
        nc.tensor.matmul(FCp[:, :], st, sb["wfc"][:, kk * O : (kk + 1) * O],
                         start=(kk == 0), stop=(kk == 7))
    outsb = consts.tile([B, O], F32, name="outsb", tag="outsb")
    nc.vector.tensor_add(outsb[:], FCp[:], sb["bfcrep"][:])
    nc.sync.dma_start(out_d.ap(), outsb[:])
    ctx.close()


_CACHE = {}


def _run(host, trace=False, n_steps=T):
    key = ("prog", n_steps)
    if key not in _CACHE:
        _CACHE[key] = build_program(host, n_steps)
    nc = _CACHE[key]
    in_map = {k: np.ascontiguousarray(v) for k, v in host.items()}
    res = run_bass_kernel_spmd(
        nc, [in_map] * NCORES, core_ids=list(range(NCORES)), trace=trace
    )
    return res


def kernel(**inputs):
    host = prepare_inputs(**{k: np.asarray(v) for k, v in inputs.items()})
    res = _run(host, trace=False)
    return np.asarray(res.results[0]["out"], np.float32)


# revision 12
# speedup vs baseline: 1.3030x; 1.0188x over previous
"""Trainium2 Bass kernel for nn_BidirRecurrentModel (2-layer bidirectional GRU).

Structure (measured 1004us baseline -> ~343us):
  * Gate PSUM split into two banks per layer-step:
      bank A [128,256] = [u | xr]   (read only by VectorE)
      bank B [128,256] = [r | z]    (read only by ScalarE)
    so ACT and DVE chain ops never touch the same PSUM bank (bank
    collisions with PE writes force Tile to serialize otherwise).
  * All matmuls emitted round-robin across the 4 tile_position column
    groups — consecutive MMs in the same 32-col group serialize on that
    PE sub-array, different groups stream concurrently.
  * h-recurrence streamed as (u,r) first then z, so sigmoid(r) starts
    as early as possible.
  * Layer-2 stream lags layer-1 by TWO steps so layer-2's x-projections
    (contracting hT1) prefetch fully off the critical path.
  * Layer-1 work emitted FIRST each tau: the engine queues are strict
    FIFO, so the serial-bottleneck loop's ops must sit at the head or
    layer-2's waiting ops head-of-line-block them (this alone was ~2x).
  * Input DMAs chunked and priority-ordered (layer-1 weights + first xT
    chunk first, tail-only tensors last) so step 0 isn't gated on the
    full ~14MB transfer; A banks double-buffered in PSUM.
  * Chain tail in bf16 (2x DVE mode), transpose output copied once.

Layouts (B=32, T=128, I=H=O=512):
  packed [128,128]: row 32k+b <-> (chunk k, batch b), col j = dim-in-chunk
  hT     [128,128]: hT[p, 32k+b] = h[b, 128k+p]
"""

import numpy as np

import concourse.bass as bass
import concourse.mybir as mybir
import concourse.tile as tile
from concourse import bacc
from concourse.bass_utils import run_bass_kernel_spmd

F32 = mybir.dt.float32
BF16 = mybir.dt.bfloat16
AF = mybir.ActivationFunctionType

B, T, I, H, O = 32, 128, 512, 512, 512
KC = 4
NCORES = 8

import ml_dtypes
BFNP = ml_dtypes.bfloat16


def _to_bf16(a):
    return np.asarray(a, np.float32).astype(BFNP)


def _pack_vec(v):
    """[512] -> packed replicated [128, 128]: out[32k+b, j] = v[128k+j]."""
    v = np.asarray(v, np.float32).reshape(KC, 128)
    out = np.repeat(v[:, None, :], B, axis=1)
    return out.reshape(128, 128)


def _blocks128(W):
    """W [512, 512] -> [128, KC*4*128]: block (k, gc) = W[128k:.., 128gc:..]."""
    out = np.zeros((128, KC * 4 * 128), np.float32)
    for k in range(KC):
        for gc in range(4):
            out[:, (k * 4 + gc) * 128 : (k * 4 + gc) * 128 + 128] = W[
                128 * k : 128 * k + 128, 128 * gc : 128 * gc + 128
            ]
    return _to_bf16(out)


def _blocks256(W0, W1):
    """Two [512,512] -> [128, KC*4*256]: block (k,gc) = [W0[k,gc] | W1[k,gc]]."""
    out = np.zeros((128, KC * 4 * 256), np.float32)
    for k in range(KC):
        for gc in range(4):
            base = (k * 4 + gc) * 256
            out[:, base : base + 128] = W0[128 * k : 128 * k + 128, 128 * gc : 128 * gc + 128]
            out[:, base + 128 : base + 256] = W1[128 * k : 128 * k + 128, 128 * gc : 128 * gc + 128]
    return _to_bf16(out)


def _pack_T(xt):
    """[B, 512] -> stationary image [128, 128]: out[p, 32k+b] = xt[b, 128k+p]."""
    a = np.asarray(xt, np.float32).T.reshape(KC, 128, B)  # [k, p, b]
    return a.transpose(1, 0, 2).reshape(128, KC * B)


def _cell_tiles(Wxh, bxh, Whh, bhh, Wxr, bxr, Whr, bhr, with_h):
    """Host tiles for one GRU cell. z = first H cols of Whh/Wxh, r = second."""
    Wz, Wr = np.asarray(Whh, np.float32)[:, :H], np.asarray(Whh, np.float32)[:, H:]
    Wxz, Wxr_h = np.asarray(Wxh, np.float32)[:, :H], np.asarray(Wxh, np.float32)[:, H:]
    bz = np.asarray(bxh, np.float32)[:H] + np.asarray(bhh, np.float32)[:H]
    br = np.asarray(bxh, np.float32)[H:] + np.asarray(bhh, np.float32)[H:]
    t = {}
    t["cgA"] = _to_bf16(
        np.concatenate([_pack_vec(np.asarray(bhr, np.float32)), _pack_vec(np.asarray(bxr, np.float32))], axis=1)
    )  # [128, 256] = [u-bias | xr-bias]
    t["cgB"] = _to_bf16(np.concatenate([_pack_vec(br), _pack_vec(bz)], axis=1))  # [r | z]
    t["wxA"] = _blocks128(np.asarray(Wxr, np.float32))        # xr  -> A[128:256]
    t["wxB"] = _blocks256(Wxr_h, Wxz)                          # r|z -> B[0:256]
    if with_h:
        t["wgA"] = _blocks128(np.asarray(Whr, np.float32))     # u -> A[0:128]
        t["wgBr"] = _blocks128(Wr)                              # r -> B[0:128]
        t["wgBz"] = _blocks128(Wz)                              # z -> B[128:256]
    return t


def prepare_inputs(x, Wxh, bxh, Whh, bhh, Wxr, bxr, Whr, bhr, Wfc, bfc):
    h = {}
    xT = np.zeros((128, T * 128), np.float32)
    for t in range(T):
        xT[:, t * 128 : (t + 1) * 128] = _pack_T(x[:, t, :])
    h["xT"] = _to_bf16(xT)
    h["xrevT"] = _to_bf16(_pack_T(x[:, T - 1, :]))

    for l in range(2):
        fw = _cell_tiles(Wxh[l, 0], bxh[l, 0], Whh[l, 0], bhh[l, 0],
                         Wxr[l, 0], bxr[l, 0], Whr[l, 0], bhr[l, 0], with_h=True)
        for k, v in fw.items():
            h[f"{k}{l}"] = v
        rv = _cell_tiles(Wxh[l, 1], bxh[l, 1], Whh[l, 1], bhh[l, 1],
                         Wxr[l, 1], bxr[l, 1], Whr[l, 1], bhr[l, 1], with_h=False)
        for k, v in rv.items():
            h[f"{k}{l}r"] = v

    wfc = np.zeros((128, 8 * O), np.float32)
    for kk in range(8):
        wfc[:, kk * O : (kk + 1) * O] = np.asarray(Wfc, np.float32)[128 * kk : 128 * (kk + 1), :]
    h["wfc"] = _to_bf16(wfc)
    h["bfcrep"] = np.repeat(np.asarray(bfc, np.float32)[None, :], B, axis=0)
    h["ideye"] = _to_bf16(np.eye(128, dtype=np.float32))
    return h


def build_program(host, n_steps=T, split_waits=False):
    nc = bacc.Bacc("TRN2", target_bir_lowering=False, debug=False, num_devices=NCORES)
    dram = {}
    for name, arr in host.items():
        dt = BF16 if arr.dtype == BFNP else F32
        dram[name] = nc.dram_tensor(name, list(arr.shape), dt, kind="ExternalInput")
    out_d = nc.dram_tensor("out", [B, O], F32, kind="ExternalOutput")

    with tile.TileContext(nc) as tc:
        _emit(tc, dram, out_d, n_steps)
    nc.compile()
    if split_waits:
        _split_multi_waits(nc)
    return nc


def _split_multi_waits(nc):
    n_nop = 0
    for fn in nc.m.functions:
        for blk in fn.blocks:
            out = []
            changed = False
            for inst in blk.instructions:
                si = inst.sync_info
                if si is not None and si.on_wait and len(si.on_wait) > 1:
                    waits = list(si.on_wait)
                    for w in waits[:-1]:
                        n_nop += 1
                        out.append(
                            mybir.InstNoOp(
                                name=f"waitnop-{n_nop}",
                                engine=inst.engine,
                                ins=[],
                                outs=[],
                                sync_info=mybir.SyncInfo(on_wait=[w], on_update=[]),
                            )
                        )
                    inst = inst.__replace__(
                        sync_info=mybir.SyncInfo(
                            on_wait=[waits[-1]], on_update=list(si.on_update or [])
                        )
                    )
                    changed = True
                out.append(inst)
            if changed:
                blk.instructions = out


def _emit(tc, dram, out_d, n_steps):
    nc = tc.nc
    from contextlib import ExitStack

    ctx = ExitStack()
    consts = ctx.enter_context(tc.tile_pool(name="consts", bufs=1))
    hpool = ctx.enter_context(tc.tile_pool(name="h", bufs=4))
    chain = ctx.enter_context(tc.tile_pool(name="chain", bufs=3))
    # A0/A1/B1 double-buffered; B0 single; one shared transpose bank.
    # 3*2 + 1 + 1 = 8 PSUM banks; FC rides tag A0 at the very end.
    psumsA = ctx.enter_context(tc.tile_pool(name="psumA", bufs=2, space="PSUM"))
    psums = ctx.enter_context(tc.tile_pool(name="psum", bufs=1, space="PSUM"))

    sb = {}

    def load(name, n_chunks=1):
        """Chunked prioritized DRAM->SBUF load: compute that depends only on
        early columns starts after the first chunk, not the whole tensor."""
        d = dram[name]
        t = consts.tile(list(d.shape), d.dtype, name=f"sb_{name}", tag=name)
        cols = d.shape[-1]
        cw = cols // n_chunks
        for c in range(n_chunks):
            sl = slice(c * cw, (c + 1) * cw)
            nc.sync.dma_start(t[:, sl], d.ap()[:, sl])
        sb[name] = t

    # priority order: what gates step 0 first, tail-only tensors last
    load("ideye")
    for nm in ("cgA0", "cgB0"):
        load(nm)
    for nm in ("wxA0", "wgA0", "wgBr0", "wgBz0"):
        load(nm, 2)
    load("wxB0", 4)
    load("xT", 16)
    for nm in ("cgA1", "cgB1"):
        load(nm)
    for nm in ("wxA1", "wgA1", "wgBr1", "wgBz1"):
        load(nm, 2)
    load("wxB1", 4)
    for nm in ("xrevT", "cgA0r", "cgB0r", "wxA0r", "cgA1r", "cgB1r", "wxA1r",
               "bfcrep"):
        load(nm)
    load("wxB0r", 2)
    load("wxB1r", 2)
    load("wfc", 4)

    ideye = sb["ideye"]

    def x_phase(l, sfx, xstat, xoff, with_h):
        """Emit consts + x-projection MMs into fresh A/B banks."""
        A = psumsA.tile([128, 256], F32, name=f"A{l}{sfx}", tag=f"A{l}")
        bpool = psumsA if l == 1 else psums
        Bk = bpool.tile([128, 256], F32, name=f"B{l}{sfx}", tag=f"B{l}")
        cgA, cgB = sb[f"cgA{l}{sfx}"], sb[f"cgB{l}{sfx}"]
        wxA, wxB = sb[f"wxA{l}{sfx}"], sb[f"wxB{l}{sfx}"]
        for gc in range(4):
            o = slice(32 * gc, 32 * gc + 32)
            tp = (0, 32 * gc)
            nc.tensor.matmul(A[o, 0:256], ideye[:, o], cgA[:, :], start=True,
                             stop=False, tile_position=tp)
            nc.tensor.matmul(Bk[o, 0:256], ideye[:, o], cgB[:, :], start=True,
                             stop=False, tile_position=tp)
        last = not with_h
        for k in range(KC):
            st = xstat[:, xoff + 32 * k : xoff + 32 * k + 32]
            for gc in range(4):
                o = slice(32 * gc, 32 * gc + 32)
                tp = (0, 32 * gc)
                stp = last and k == KC - 1
                nc.tensor.matmul(A[o, 128:256], st,
                                 wxA[:, (k * 4 + gc) * 128 : (k * 4 + gc) * 128 + 128],
                                 start=False, stop=stp, tile_position=tp)
                nc.tensor.matmul(Bk[o, 0:256], st,
                                 wxB[:, (k * 4 + gc) * 256 : (k * 4 + gc) * 256 + 256],
                                 start=False, stop=stp, tile_position=tp)
        return A, Bk

    def h_phase(l, A, Bk, hTp):
        """u,r first (so sigmoid(r) starts early), then z."""
        wgA, wgBr, wgBz = sb[f"wgA{l}"], sb[f"wgBr{l}"], sb[f"wgBz{l}"]
        for k in range(KC):
            st = hTp[:, 32 * k : 32 * k + 32]
            for gc in range(4):
                o = slice(32 * gc, 32 * gc + 32)
                tp = (0, 32 * gc)
                blk = slice((k * 4 + gc) * 128, (k * 4 + gc) * 128 + 128)
                nc.tensor.matmul(A[o, 0:128], st, wgA[:, blk], start=False,
                                 stop=(k == KC - 1), tile_position=tp)
                nc.tensor.matmul(Bk[o, 0:128], st, wgBr[:, blk], start=False,
                                 stop=False, tile_position=tp)
        for k in range(KC):
            st = hTp[:, 32 * k : 32 * k + 32]
            for gc in range(4):
                o = slice(32 * gc, 32 * gc + 32)
                tp = (0, 32 * gc)
                blk = slice((k * 4 + gc) * 128, (k * 4 + gc) * 128 + 128)
                nc.tensor.matmul(Bk[o, 128:256], st, wgBz[:, blk], start=False,
                                 stop=(k == KC - 1), tile_position=tp)

    def chain_phase(l, tag, A, Bk, hn_prev):
        rs = chain.tile([128, 128], F32, name=f"rs{tag}", tag=f"rs{tag}")
        nc.scalar.activation(rs[:], Bk[:, 0:128], AF.Sigmoid)
        zs = chain.tile([128, 128], BF16, name=f"zs{tag}", tag=f"zs{tag}")
        nc.scalar.activation(zs[:], Bk[:, 128:256], AF.Sigmoid)
        v0 = chain.tile([128, 128], F32, name=f"v0{tag}", tag=f"v0{tag}")
        nc.vector.tensor_mul(v0[:], A[:, 0:128], rs[:])
        v1 = chain.tile([128, 128], BF16, name=f"v1{tag}", tag=f"v1{tag}")
        nc.vector.tensor_add(v1[:], v0[:], A[:, 128:256])
        if hn_prev is not None:
            p = chain.tile([128, 128], BF16, name=f"p{tag}", tag=f"p{tag}")
            nc.vector.tensor_mul(p[:], zs[:], hn_prev[:])
        g = chain.tile([128, 128], BF16, name=f"g{tag}", tag=f"g{tag}")
        nc.scalar.activation(g[:], v1[:], AF.Tanh)
        # qn = (zs - 1) * g = -(1-z)*g in ONE fused DVE op (drops the
        # sigmoid(-z) ACT op from the per-step chain entirely)
        qn = chain.tile([128, 128], BF16, name=f"q{tag}", tag=f"q{tag}")
        nc.vector.scalar_tensor_tensor(
            qn[:], zs[:], 1.0, g[:],
            mybir.AluOpType.subtract, mybir.AluOpType.mult,
        )
        if hn_prev is None:
            # h0 = 0: h' = (1-z)*g = -qn
            hn0 = chain.tile([128, 128], BF16, name=f"h0{tag}", tag=f"h0{tag}")
            nc.vector.tensor_scalar_mul(hn0[:], qn[:], -1.0)
            return hn0
        hn = hpool.tile([128, 128], BF16, name=f"hn{l}", tag=f"hn{l}")
        nc.vector.tensor_sub(hn[:], p[:], qn[:])
        return hn

    def t_phase(l, tag, hn):
        Tp = psums.tile([128, 128], BF16, name=f"T{tag}", tag="T")
        nc.tensor.transpose(Tp[:], hn[:], ideye[:])
        hT = hpool.tile([128, 128], BF16, name=f"hT{tag}", tag=f"hT{l}")
        nc.scalar.copy(hT[:], Tp[:])
        return hT

    # ---------------- forward recurrence ----------------
    # L1 (layer 0 fwd) at step t1 = tau; L2 (layer 1 fwd) lags two steps.
    AB1 = {}
    AB2 = {}
    hT1 = {}  # step -> tile (kept ~2 taus)
    hT2 = {}
    hn1 = hn2 = None

    AB1[0] = x_phase(0, "", sb["xT"], 0, with_h=False)

    for tau in range(n_steps + 2):
        t1 = tau
        t2 = tau - 2
        # L1 first everywhere: it is the serial bottleneck loop (its state
        # feeds both its own next step and L2's input stream).
        if t1 < n_steps and t1 > 0:
            h_phase(0, *AB1[t1], hT1[t1 - 1])
        if 0 <= t2 < n_steps and t2 > 0:
            h_phase(1, *AB2[t2], hT2[t2 - 1])
        if t1 < n_steps:
            hn1 = chain_phase(0, "1", *AB1[t1], hn1 if t1 > 0 else None)
        if t1 + 1 < n_steps:
            AB1[t1 + 1] = x_phase(0, "", sb["xT"], (t1 + 1) * 128, with_h=(t1 + 1) > 0)
        if 0 <= t2 < n_steps:
            hn2 = chain_phase(1, "2", *AB2[t2], hn2 if t2 > 0 else None)
        # x2 BEFORE t_phase(L1): keeps L2's x-stream out of the PE FIFO
        # segment between L1's transpose and the next tau's L1 matmuls
        # (the serial-bottleneck loop). Its input hT1[t2+1] is from the
        # previous tau, and B1 is double-buffered so it cannot stall.
        if 0 <= t2 + 1 < n_steps:
            AB2[t2 + 1] = x_phase(1, "", hT1[t2 + 1], 0, with_h=(t2 + 1) > 0)
        if t1 < n_steps:
            hT1[t1] = t_phase(0, f"1_{t1}", hn1)
            hT1.pop(t1 - 3, None)
        if 0 <= t2 < n_steps:
            hT2[t2] = t_phase(1, f"2_{t2}", hn2)
            hT2.pop(t2 - 2, None)
        AB1.pop(t1, None)
        AB2.pop(t2, None)

    # ---------------- reverse stream: one step per layer, h0 = 0 ----------------
    Ar, Br = x_phase(0, "r", sb["xrevT"], 0, with_h=False)
    q1r = chain_phase(0, "r1", Ar, Br, None)
    h1rT = t_phase(0, "r1", q1r)
    Ar2, Br2 = x_phase(1, "r", h1rT, 0, with_h=False)
    q2r = chain_phase(1, "r2", Ar2, Br2, None)
    h2rT = t_phase(1, "r2", q2r)

    # ---------------- final FC ----------------
    hT2_last = hT2[n_steps - 1]
    FCp = psumsA.tile([B, O], F32, name="FCp", tag="A0")
    for kk in range(8):
        st = hT2_last[:, 32 * kk : 32 * kk + 32] if kk < 4 else h2rT[:, 32 * (kk - 4) : 32 * (kk - 4) + 32]
        nc.tensor.matmul(FCp[:, :], st, sb["wfc"][:, kk * O : (kk + 1) * O],
                         start=(kk == 0), stop=(kk == 7))
    outsb = consts.tile([B, O], F32, name="outsb", tag="outsb")
    nc.vector.tensor_add(outsb[:], FCp[:], sb["bfcrep"][:])
    nc.sync.dma_start(out_d.ap(), outsb[:])
    ctx.close()


_CACHE = {}


def _run(host, trace=False, n_steps=T):
    key = ("prog", n_steps)
    if key not in _CACHE:
        _CACHE[key] = build_program(host, n_steps)
    nc = _CACHE[key]
    in_map = {k: np.ascontiguousarray(v) for k, v in host.items()}
    res = run_bass_kernel_spmd(
        nc, [in_map] * NCORES, core_ids=list(range(NCORES)), trace=trace
    )
    return res


def kernel(**inputs):
    host = prepare_inputs(**{k: np.asarray(v) for k, v in inputs.items()})
    res = _run(host, trace=False)
    return np.asarray(res.results[0]["out"], np.float32)
